# revision 13
# baseline (speedup 1.0000x reference)
"""CTRNN cell (6 Euler unfolds) on 8 Trainium2 NeuronCores.

Math (per unfold, 6x):
    f     = tanh([x, s] @ W + b)
    s_new = s + 0.1 * (-s + f)  = 0.9*s + 0.1*f

Strategy (v4 — chunk-pipelined bf16, TT-subtract critical path):
  - Data-parallel over batch: B=8192 -> 1024 rows/core, no cross-core
    communication. Host does the cheap numpy transposes/packing.
  - Everything transposed on-chip (feature dim on SBUF partitions, batch
    on the free dim); W slices are directly the stationary lhsT.
  - pre = x @ W_top computed once; per-unfold matmuls in *delta* form:
    one persistent PSUM accumulator per (m-tile, chunk) holds
    pre + s_k @ W_bot across all unfolds via psum += (f_k - s_k) @ (0.1*W_bot).
    7-logical-matmul FLOP floor, PSUM never restarts.
  - bf16 everywhere except PSUM and the f32 output: x, W, s, f, tmp.
    PSUM accumulation stays fp32. Errors are deterministic (fixed seed):
    ~2.6e-2 absmax vs the 6.3e-2 gate.
  - The state is plain s (bf16, unscaled) so the critical-path op
    tmp = f - s is a dual-pumped all-bf16 DVE tensor_tensor (334 ns per
    512-chunk; scalar_tensor_tensor does NOT dual-pump, measured).  The
    m3 subtract (the last-consumed k-tile, ~3us of slack) runs on GpSimd
    (~1.4us there; stt is ISA-illegal on Pool so the s += 0.1*tmp updates
    all stay on DVE). Per chunk-slot: PE 3.63us (pacer), ACT 2.5us,
    DVE ~3.4us, GpSimd ~1.4us.
  - Init is ONE 8-k-tile round per chunk alternating x/s k-tiles, so it
    consumes input pieces in exactly DMA-arrival order: x on the sync
    HWDGE ring, W_top/W_bot interleaved on the scalar ring, s on SWDGE,
    each ring issuing pieces in need order. 0.1*W_bot for the delta
    rounds is derived on-chip (ACT copy, scale=0.1) per landed piece.
  - Chunk pipelining: each 1024-batch round is split into two 512-batch
    chunk-rounds (c0/c1); tanh/subtract for chunk c0 run while the PE
    does chunk c1, so the tensor engine never waits at unfold bounds.
  - The final unfold is uniform with the others: the updated bf16 state
    itself is the output, DMA'd out per (m, chunk) piece as soon as its
    stt lands (1 MB total); the host just transposes/upcasts.
  - A minimal junk-matmul + junk-tanh warm-up starts the HAM un-throttle
    clock and pre-loads the ACT tanh table during the DMA lead-in.
"""

import numpy as np

UNFOLDS = 6
DT = 0.1
B, D, N = 8192, 512, 512
NCORES = 8
BC = B // NCORES          # batch rows per core
CH = 512                  # chunk: matmul moving free dim (1 PSUM bank)
NCH = BC // CH            # 2
P = 128
KT = D // P               # k-tiles (4) for each of W_top / W_bot
MT = N // P               # m-tiles of the output dim (4)

_compiled_nc = None


def _build_nc():
    import concourse.bass as bass  # noqa: F401
    import concourse.bacc as bacc
    import concourse.tile as tile
    from concourse import mybir

    f32 = mybir.dt.float32
    bf16 = mybir.dt.bfloat16
    MULT = mybir.AluOpType.mult
    ADD = mybir.AluOpType.add
    SUB = mybir.AluOpType.subtract
    TANH = mybir.ActivationFunctionType.Tanh

    nc = bacc.Bacc("TRN2", target_bir_lowering=False, debug=False)

    xB = nc.dram_tensor("xB", [P, KT * BC], bf16, kind="ExternalInput").ap()
    sB = nc.dram_tensor("sB", [P, KT * BC], bf16, kind="ExternalInput").ap()
    wB = nc.dram_tensor("wB", [P, 2 * KT * N], bf16, kind="ExternalInput").ap()
    bias = nc.dram_tensor("bias", [N], f32, kind="ExternalInput").ap()
    outT = nc.dram_tensor("outT", [P, KT * BC], bf16, kind="ExternalOutput").ap()

    with tile.TileContext(nc) as tc:
        with (
            tc.tile_pool(name="weights", bufs=1) as wpool,
            tc.tile_pool(name="data", bufs=1) as data,
            tc.tile_pool(name="tmp", bufs=2) as tmpp,
            tc.tile_pool(name="fpool", bufs=2) as fpool,
            tc.tile_pool(name="opool", bufs=1) as opool,
            tc.tile_pool(name="psum", bufs=1, space="PSUM") as psump,
        ):
            # ---- warm-up: start HAM clock + load the ACT tanh table ------
            junk = wpool.tile([P, 2 * P], bf16, tag="junk", name="junk")
            nc.vector.memset(junk[:], 0)
            junk2 = wpool.tile([P, 2 * P], bf16, tag="junk2", name="junk2")

            # ---- input DMAs: need-ordered pieces on 3 rings --------------
            x_sb = data.tile([P, KT * BC], bf16, tag="x", name="x_sb")
            v_sb = data.tile([P, KT * BC], bf16, tag="v", name="v_sb")
            w_sb = wpool.tile([P, 2 * KT * N], bf16, tag="w", name="w_sb")
            wb01_sb = wpool.tile([P, KT * N], bf16, tag="wb01", name="wb01_sb")
            bias_sb = wpool.tile([P, MT], f32, tag="bias", name="bias_sb")

            def xsl(j, c):
                # c-outer packing: chunk c occupies one contiguous half
                return slice(c * KT * CH + j * CH, c * KT * CH + (j + 1) * CH)

            HC = KT * CH  # elements per chunk-half (2048)
            # coarse 512 KB pieces; rings drain round-robin at packet
            # granularity, so the per-ring FIFO orders below give a global
            # arrival order matching init consumption:
            #   [x_c0, W_A, s_c0], [x_c1, W_B, bias], [s_c1]
            nc.sync.dma_start(x_sb[:, 0:HC], xB[:, 0:HC])
            nc.sync.dma_start(x_sb[:, HC:2 * HC], xB[:, HC:2 * HC])
            nc.sync.dma_start(v_sb[:, HC:2 * HC], sB[:, HC:2 * HC])
            nc.scalar.dma_start(w_sb[:, 0:HC], wB[:, 0:HC])
            nc.scalar.dma_start(w_sb[:, HC:2 * HC], wB[:, HC:2 * HC])
            nc.gpsimd.dma_start(v_sb[:, 0:HC], sB[:, 0:HC])
            nc.gpsimd.dma_start(bias_sb[:], bias.rearrange("(m p) -> p m", p=P))
            # junk tanh pre-loads the ACT tanh table (~2.7us) during the W
            # transfer; emitted AFTER the W dma_starts so it does not delay
            # their dispatch on the scalar sequencer.
            nc.scalar.activation(junk2[:], junk[:], TANH)
            # 0.1*W_bot for the delta rounds (ACT copy, scale=0.1)
            for j in range(KT):
                nc.scalar.mul(wb01_sb[:, j * N:(j + 1) * N],
                              w_sb[:, j * BC + CH: (j + 1) * BC], DT)

            # ---- persistent PSUM accumulators ----------------------------
            ps = [psump.tile([P, BC], f32, tag=f"ps{m}", name=f"ps{m}")
                  for m in range(MT)]

            # junk matmuls keep the PE busy (~3.4us continuous) so HAM
            # un-throttles to 2.4 GHz before the real matmuls stream.
            # 16 x 256-wide to 8 distinct half-banks: no WAW chains, so
            # they issue back-to-back; overwritten by the first start=True
            # real matmul per half-bank.
            for r in range(32):
                m, c = r % MT, (r // MT) % NCH
                nc.tensor.matmul(
                    ps[m][:, c * CH: c * CH + 2 * P],
                    lhsT=junk[:, 0:P], rhs=junk[:],
                    start=True, stop=True, skip_group_check=True,
                )

            # init: psum = x @ W_top + s0 @ W_bot, one 8-k-tile round per
            # chunk, alternating x/s k-tiles (DMA-arrival order)
            for c in range(NCH):
                INIT_ORDER = [("x", 0), ("x", 1), ("s", 0), ("s", 1),
                              ("x", 2), ("x", 3), ("s", 2), ("s", 3)]
                for t, (kind, j) in enumerate(INIT_ORDER):
                    off = j * BC + (0 if kind == "x" else CH)
                    rhs_t = x_sb if kind == "x" else v_sb
                    for m in range(MT):
                        nc.tensor.matmul(
                            ps[m][:, c * CH:(c + 1) * CH],
                            lhsT=w_sb[:, off + m * P: off + (m + 1) * P],
                            rhs=rhs_t[:, xsl(j, c)],
                            start=(t == 0),
                            stop=False,
                            skip_group_check=True,
                        )

            # ---- unfolds (chunk-pipelined) -------------------------------
            # Per chunk-round:
            #   f = tanh(psum + bias)            (ACT, 512 wide, bf16 out)
            #   tmp = f - s                      (TT sub: DVE m0/m1, GpSimd m2/m3)
            #   psum += tmp @ (0.1*W_bot)        (16 matmuls; skipped on k=5)
            #   s += 0.1*tmp                     (stt, DVE)
            # The last unfold is uniform: the updated bf16 state IS the
            # output, DMA'd per (m, chunk) piece (host just unpacks).
            out_eng = [nc.sync, nc.scalar]
            for k in range(UNFOLDS - 1):
                last = False
                tmp_t = [tmpp.tile([P, BC], bf16, tag=f"tmp{j}",
                                   name=f"tmp{k}_{j}")
                         for j in range(MT)]
                t01_t = [tmpp.tile([P, BC], bf16, tag=f"t01_{j}",
                                   name=f"t01_{k}_{j}")
                         for j in range(2)]
                f_t = [fpool.tile([P, BC], bf16, tag=f"f{m}", name=f"f{k}_{m}")
                       for m in range(MT)]
                for c in range(NCH):
                    cs = slice(c * CH, (c + 1) * CH)
                    for m in range(MT):
                        nc.scalar.activation(
                            f_t[m][:, cs], ps[m][:, cs], TANH,
                            bias=bias_sb[:, m:m + 1], scale=1.0,
                        )
                    for m in range(MT):
                        nc.vector.tensor_tensor(
                            tmp_t[m][:, cs], f_t[m][:, cs],
                            v_sb[:, xsl(m, c)], SUB,
                        )
                    # m-outer: ps[m] only written after tanh(m) freed it,
                    # and m0 closes early for the next slot's tanh chain
                    for m in range(MT):
                        for j in range(KT):
                            nc.tensor.matmul(
                                ps[m][:, cs],
                                lhsT=wb01_sb[:, j * N + m * P:
                                             j * N + (m + 1) * P],
                                rhs=tmp_t[j][:, cs],
                                start=False,
                                stop=(k == UNFOLDS - 2 and j == KT - 1),
                                skip_group_check=True,
                            )
                    # state update (off the critical path): ACT scales
                    # 0.1*tmp for m0/m1 (slack capacity there), making the
                    # add dual-pumped; m2/m3 use the 1x stt on DVE.
                    ncop = 2 if k < UNFOLDS - 2 else 0
                    for m in range(ncop):
                        nc.scalar.mul(t01_t[m][:, cs], tmp_t[m][:, cs], DT)
                        nc.vector.tensor_tensor(
                            v_sb[:, xsl(m, c)], v_sb[:, xsl(m, c)],
                            t01_t[m][:, cs], ADD,
                        )
                    for m in range(ncop, MT):
                        nc.vector.scalar_tensor_tensor(
                            v_sb[:, xsl(m, c)], tmp_t[m][:, cs], DT,
                            v_sb[:, xsl(m, c)], op0=MULT, op1=ADD,
                        )

            # final unfold: out10 = 9*s + f = 10*s_6 (bf16, in place over
            # the state), one strided DMA per chunk; host scales by 0.1.
            f_t = [fpool.tile([P, BC], bf16, tag=f"f{m}", name=f"f5_{m}")
                   for m in range(MT)]
            for c in range(NCH):
                cs = slice(c * CH, (c + 1) * CH)
                for m in range(MT):
                    nc.scalar.activation(
                        f_t[m][:, cs], ps[m][:, cs], TANH,
                        bias=bias_sb[:, m:m + 1], scale=1.0,
                    )
                for m in range(MT):
                    nc.vector.scalar_tensor_tensor(
                        v_sb[:, xsl(m, c)], v_sb[:, xsl(m, c)], 9.0,
                        f_t[m][:, cs], op0=MULT, op1=ADD,
                    )
                out_eng[c].dma_start(outT[:, c * HC:(c + 1) * HC],
                                     v_sb[:, c * HC:(c + 1) * HC])

    nc.compile()
    return nc


def _get_nc():
    global _compiled_nc
    if _compiled_nc is None:
        _compiled_nc = _build_nc()
    return _compiled_nc


def _couter_pack(a):
    """(KT*P, BC) -> (P, [c][j][CH]): chunk-outer, k-tiles inside."""
    return np.ascontiguousarray(
        a.reshape(KT, P, NCH, CH).transpose(1, 2, 0, 3).reshape(P, -1))


def make_in_maps(x, s, W, b):
    """Shard + pack host-side. x/W/s in bf16; all packed as (128, KT*free)
    k-tile layouts so every DMA piece has contiguous per-partition runs."""
    import ml_dtypes
    bf16 = ml_dtypes.bfloat16

    xT = np.ascontiguousarray(x.T)            # (D, B)
    sT = np.ascontiguousarray(s.T)            # (N, B)
    # W interleaved: (P, [j][Wt_j(512) | Wb_j(512)])
    Wt = W[:D].astype(bf16).reshape(KT, P, 1, N)
    Wb = W[D:].astype(bf16).reshape(KT, P, 1, N)
    Wi = np.ascontiguousarray(
        np.concatenate([Wt, Wb], axis=2).transpose(1, 0, 2, 3).reshape(P, -1))
    in_maps = []
    for c in range(NCORES):
        sl = slice(c * BC, (c + 1) * BC)
        in_maps.append({
            "xB": _couter_pack(xT[:, sl].astype(bf16)),
            "sB": _couter_pack(sT[:, sl].astype(bf16)),
            "wB": Wi,
            "bias": b,
        })
    return in_maps


def kernel(**inputs):
    from concourse.bass_utils import run_bass_kernel_spmd

    x = np.asarray(inputs["inputs"], dtype=np.float32)
    s = np.asarray(inputs["state"], dtype=np.float32)
    W = np.ascontiguousarray(np.asarray(inputs["W"], dtype=np.float32))
    b = np.ascontiguousarray(np.asarray(inputs["bias"], dtype=np.float32))

    in_maps = make_in_maps(x, s, W, b)
    nc = _get_nc()
    res = run_bass_kernel_spmd(nc, in_maps, list(range(NCORES))).results
    # unpack (P, [c][j][CH]) per core -> (B, N)
    parts = []
    for c in range(NCORES):
        o = res[c]["outT"].astype(np.float32).reshape(P, NCH, KT, CH)
        parts.append(o.transpose(1, 3, 2, 0).reshape(BC, N))
    out = np.ascontiguousarray(DT * np.concatenate(parts, axis=0))
    return (out, out)


# revision 14
# speedup vs baseline: 1.0258x; 1.0258x over previous
"""CTRNN cell (6 Euler unfolds) on 8 Trainium2 NeuronCores.

Math (per unfold, 6x):
    f     = tanh([x, s] @ W + b)
    s_new = s + 0.1 * (-s + f)  = 0.9*s + 0.1*f

Strategy (v4 — chunk-pipelined bf16, TT-subtract critical path):
  - Data-parallel over batch: B=8192 -> 1024 rows/core, no cross-core
    communication. Host does the cheap numpy transposes/packing.
  - Everything transposed on-chip (feature dim on SBUF partitions, batch
    on the free dim); W slices are directly the stationary lhsT.
  - pre = x @ W_top computed once; per-unfold matmuls in *delta* form:
    one persistent PSUM accumulator per (m-tile, chunk) holds
    pre + s_k @ W_bot across all unfolds via psum += (f_k - s_k) @ (0.1*W_bot).
    7-logical-matmul FLOP floor, PSUM never restarts.
  - bf16 everywhere except PSUM and the f32 output: x, W, s, f, tmp.
    PSUM accumulation stays fp32. Errors are deterministic (fixed seed):
    ~2.6e-2 absmax vs the 6.3e-2 gate.
  - The state is plain s (bf16, unscaled) so the critical-path op
    tmp = f - s is a dual-pumped all-bf16 DVE tensor_tensor (334 ns per
    512-chunk; scalar_tensor_tensor does NOT dual-pump, measured).  The
    m3 subtract (the last-consumed k-tile, ~3us of slack) runs on GpSimd
    (~1.4us there; stt is ISA-illegal on Pool so the s += 0.1*tmp updates
    all stay on DVE). Per chunk-slot: PE 3.63us (pacer), ACT 2.5us,
    DVE ~3.4us, GpSimd ~1.4us.
  - Init is ONE 8-k-tile round per chunk alternating x/s k-tiles, so it
    consumes input pieces in exactly DMA-arrival order: x on the sync
    HWDGE ring, W_top/W_bot interleaved on the scalar ring, s on SWDGE,
    each ring issuing pieces in need order. 0.1*W_bot for the delta
    rounds is derived on-chip (ACT copy, scale=0.1) per landed piece.
  - Chunk pipelining: each 1024-batch round is split into two 512-batch
    chunk-rounds (c0/c1); tanh/subtract for chunk c0 run while the PE
    does chunk c1, so the tensor engine never waits at unfold bounds.
  - The final unfold is uniform with the others: the updated bf16 state
    itself is the output, DMA'd out per (m, chunk) piece as soon as its
    stt lands (1 MB total); the host just transposes/upcasts.
  - A minimal junk-matmul + junk-tanh warm-up starts the HAM un-throttle
    clock and pre-loads the ACT tanh table during the DMA lead-in.
"""

import numpy as np

UNFOLDS = 6
DT = 0.1
B, D, N = 8192, 512, 512
NCORES = 8
BC = B // NCORES          # batch rows per core
CH = 512                  # chunk: matmul moving free dim (1 PSUM bank)
NCH = BC // CH            # 2
P = 128
KT = D // P               # k-tiles (4) for each of W_top / W_bot
MT = N // P               # m-tiles of the output dim (4)

_compiled_nc = None


def _build_nc():
    import concourse.bass as bass  # noqa: F401
    import concourse.bacc as bacc
    import concourse.tile as tile
    from concourse import mybir

    f32 = mybir.dt.float32
    bf16 = mybir.dt.bfloat16
    MULT = mybir.AluOpType.mult
    ADD = mybir.AluOpType.add
    SUB = mybir.AluOpType.subtract
    TANH = mybir.ActivationFunctionType.Tanh

    nc = bacc.Bacc("TRN2", target_bir_lowering=False, debug=False)

    xB = nc.dram_tensor("xB", [P, KT * BC], bf16, kind="ExternalInput").ap()
    sB = nc.dram_tensor("sB", [P, KT * BC], bf16, kind="ExternalInput").ap()
    wB = nc.dram_tensor("wB", [P, 2 * KT * N], bf16, kind="ExternalInput").ap()
    bias = nc.dram_tensor("bias", [N], f32, kind="ExternalInput").ap()
    outT = nc.dram_tensor("outT", [P, KT * BC], bf16, kind="ExternalOutput").ap()

    with tile.TileContext(nc) as tc:
        with (
            tc.tile_pool(name="weights", bufs=1) as wpool,
            tc.tile_pool(name="data", bufs=1) as data,
            tc.tile_pool(name="tmp", bufs=2) as tmpp,
            tc.tile_pool(name="fpool", bufs=2) as fpool,
            tc.tile_pool(name="opool", bufs=1) as opool,
            tc.tile_pool(name="psum", bufs=1, space="PSUM") as psump,
        ):
            # ---- warm-up: start HAM clock + load the ACT tanh table ------
            junk = wpool.tile([P, 2 * P], bf16, tag="junk", name="junk")
            nc.vector.memset(junk[:], 0)
            junk2 = wpool.tile([P, 2 * P], bf16, tag="junk2", name="junk2")

            # ---- input DMAs: need-ordered pieces on 3 rings --------------
            x_sb = data.tile([P, KT * BC], bf16, tag="x", name="x_sb")
            v_sb = data.tile([P, KT * BC], bf16, tag="v", name="v_sb")
            w_sb = wpool.tile([P, 2 * KT * N], bf16, tag="w", name="w_sb")
            wb01_sb = wpool.tile([P, KT * N], bf16, tag="wb01", name="wb01_sb")
            bias_sb = wpool.tile([P, MT], f32, tag="bias", name="bias_sb")

            def xsl(j, c):
                # c-outer packing: chunk c occupies one contiguous half
                return slice(c * KT * CH + j * CH, c * KT * CH + (j + 1) * CH)

            HC = KT * CH  # elements per chunk-half (2048)
            # coarse 512 KB pieces; rings drain round-robin at packet
            # granularity, so the per-ring FIFO orders below give a global
            # arrival order matching init consumption:
            #   [x_c0, W_A, s_c0], [x_c1, W_B, bias], [s_c1]
            nc.sync.dma_start(x_sb[:, 0:HC], xB[:, 0:HC])
            nc.sync.dma_start(x_sb[:, HC:2 * HC], xB[:, HC:2 * HC])
            nc.sync.dma_start(v_sb[:, HC:2 * HC], sB[:, HC:2 * HC])
            nc.scalar.dma_start(w_sb[:, 0:HC], wB[:, 0:HC])
            nc.scalar.dma_start(w_sb[:, HC:2 * HC], wB[:, HC:2 * HC])
            nc.gpsimd.dma_start(v_sb[:, 0:HC], sB[:, 0:HC])
            nc.gpsimd.dma_start(bias_sb[:], bias.rearrange("(m p) -> p m", p=P))
            # junk tanh pre-loads the ACT tanh table (~2.7us) during the W
            # transfer; emitted AFTER the W dma_starts so it does not delay
            # their dispatch on the scalar sequencer.
            nc.scalar.activation(junk2[:], junk[:], TANH)
            # 0.1*W_bot for the delta rounds (ACT copy, scale=0.1)
            for j in range(KT):
                nc.scalar.mul(wb01_sb[:, j * N:(j + 1) * N],
                              w_sb[:, j * BC + CH: (j + 1) * BC], DT)

            # ---- persistent PSUM accumulators ----------------------------
            ps = [psump.tile([P, BC], f32, tag=f"ps{m}", name=f"ps{m}")
                  for m in range(MT)]

            # junk matmuls keep the PE busy (~3.4us continuous) so HAM
            # un-throttles to 2.4 GHz before the real matmuls stream.
            # 16 x 256-wide to 8 distinct half-banks: no WAW chains, so
            # they issue back-to-back; overwritten by the first start=True
            # real matmul per half-bank.
            for r in range(26):
                m, c = r % MT, (r // MT) % NCH
                nc.tensor.matmul(
                    ps[m][:, c * CH: c * CH + 2 * P],
                    lhsT=junk[:, 0:P], rhs=junk[:],
                    start=True, stop=True, skip_group_check=True,
                )

            # init: psum = x @ W_top + s0 @ W_bot, one 8-k-tile round per
            # chunk, alternating x/s k-tiles (DMA-arrival order)
            for c in range(NCH):
                INIT_ORDER = [("x", 0), ("x", 1), ("s", 0), ("s", 1),
                              ("x", 2), ("x", 3), ("s", 2), ("s", 3)]
                for t, (kind, j) in enumerate(INIT_ORDER):
                    off = j * BC + (0 if kind == "x" else CH)
                    rhs_t = x_sb if kind == "x" else v_sb
                    for m in range(MT):
                        nc.tensor.matmul(
                            ps[m][:, c * CH:(c + 1) * CH],
                            lhsT=w_sb[:, off + m * P: off + (m + 1) * P],
                            rhs=rhs_t[:, xsl(j, c)],
                            start=(t == 0),
                            stop=False,
                            skip_group_check=True,
                        )

            # ---- unfolds (chunk-pipelined) -------------------------------
            # Per chunk-round:
            #   f = tanh(psum + bias)            (ACT, 512 wide, bf16 out)
            #   tmp = f - s                      (TT sub: DVE m0/m1, GpSimd m2/m3)
            #   psum += tmp @ (0.1*W_bot)        (16 matmuls; skipped on k=5)
            #   s += 0.1*tmp                     (stt, DVE)
            # The last unfold is uniform: the updated bf16 state IS the
            # output, DMA'd per (m, chunk) piece (host just unpacks).
            out_eng = [nc.sync, nc.scalar]
            for k in range(UNFOLDS - 1):
                last = False
                tmp_t = [tmpp.tile([P, BC], bf16, tag=f"tmp{j}",
                                   name=f"tmp{k}_{j}")
                         for j in range(MT)]
                t01_t = [tmpp.tile([P, BC], bf16, tag=f"t01_{j}",
                                   name=f"t01_{k}_{j}")
                         for j in range(2)]
                f_t = [fpool.tile([P, BC], bf16, tag=f"f{m}", name=f"f{k}_{m}")
                       for m in range(MT)]
                for c in range(NCH):
                    cs = slice(c * CH, (c + 1) * CH)
                    for m in range(MT):
                        nc.scalar.activation(
                            f_t[m][:, cs], ps[m][:, cs], TANH,
                            bias=bias_sb[:, m:m + 1], scale=1.0,
                        )
                    for m in range(MT):
                        nc.vector.tensor_tensor(
                            tmp_t[m][:, cs], f_t[m][:, cs],
                            v_sb[:, xsl(m, c)], SUB,
                        )
                    # m-outer: ps[m] only written after tanh(m) freed it,
                    # and m0 closes early for the next slot's tanh chain
                    for m in range(MT):
                        for j in range(KT):
                            nc.tensor.matmul(
                                ps[m][:, cs],
                                lhsT=wb01_sb[:, j * N + m * P:
                                             j * N + (m + 1) * P],
                                rhs=tmp_t[j][:, cs],
                                start=False,
                                stop=(k == UNFOLDS - 2 and j == KT - 1),
                                skip_group_check=True,
                            )
                    # state update (off the critical path): ACT scales
                    # 0.1*tmp for m0/m1 (slack capacity there), making the
                    # add dual-pumped; m2/m3 use the 1x stt on DVE.
                    ncop = 1 if k < UNFOLDS - 2 else 0
                    for m in range(ncop):
                        nc.scalar.mul(t01_t[m][:, cs], tmp_t[m][:, cs], DT)
                        nc.vector.tensor_tensor(
                            v_sb[:, xsl(m, c)], v_sb[:, xsl(m, c)],
                            t01_t[m][:, cs], ADD,
                        )
                    for m in range(ncop, MT):
                        nc.vector.scalar_tensor_tensor(
                            v_sb[:, xsl(m, c)], tmp_t[m][:, cs], DT,
                            v_sb[:, xsl(m, c)], op0=MULT, op1=ADD,
                        )

            # final unfold: out10 = 9*s + f = 10*s_6 (bf16, in place over
            # the state), one strided DMA per chunk; host scales by 0.1.
            f_t = [fpool.tile([P, BC], bf16, tag=f"f{m}", name=f"f5_{m}")
                   for m in range(MT)]
            for c in range(NCH):
                cs = slice(c * CH, (c + 1) * CH)
                for m in range(MT):
                    nc.scalar.activation(
                        f_t[m][:, cs], ps[m][:, cs], TANH,
                        bias=bias_sb[:, m:m + 1], scale=1.0,
                    )
                for m in range(MT):
                    nc.vector.scalar_tensor_tensor(
                        v_sb[:, xsl(m, c)], v_sb[:, xsl(m, c)], 9.0,
                        f_t[m][:, cs], op0=MULT, op1=ADD,
                    )
                out_eng[c].dma_start(outT[:, c * HC:(c + 1) * HC],
                                     v_sb[:, c * HC:(c + 1) * HC])

    nc.compile()
    return nc


def _get_nc():
    global _compiled_nc
    if _compiled_nc is None:
        _compiled_nc = _build_nc()
    return _compiled_nc


def _couter_pack(a):
    """(KT*P, BC) -> (P, [c][j][CH]): chunk-outer, k-tiles inside."""
    return np.ascontiguousarray(
        a.reshape(KT, P, NCH, CH).transpose(1, 2, 0, 3).reshape(P, -1))


def make_in_maps(x, s, W, b):
    """Shard + pack host-side. x/W/s in bf16; all packed as (128, KT*free)
    k-tile layouts so every DMA piece has contiguous per-partition runs."""
    import ml_dtypes
    bf16 = ml_dtypes.bfloat16

    xT = np.ascontiguousarray(x.T)            # (D, B)
    sT = np.ascontiguousarray(s.T)            # (N, B)
    # W interleaved: (P, [j][Wt_j(512) | Wb_j(512)])
    Wt = W[:D].astype(bf16).reshape(KT, P, 1, N)
    Wb = W[D:].astype(bf16).reshape(KT, P, 1, N)
    Wi = np.ascontiguousarray(
        np.concatenate([Wt, Wb], axis=2).transpose(1, 0, 2, 3).reshape(P, -1))
    in_maps = []
    for c in range(NCORES):
        sl = slice(c * BC, (c + 1) * BC)
        in_maps.append({
            "xB": _couter_pack(xT[:, sl].astype(bf16)),
            "sB": _couter_pack(sT[:, sl].astype(bf16)),
            "wB": Wi,
            "bias": b,
        })
    return in_maps


def kernel(**inputs):
    from concourse.bass_utils import run_bass_kernel_spmd

    x = np.asarray(inputs["inputs"], dtype=np.float32)
    s = np.asarray(inputs["state"], dtype=np.float32)
    W = np.ascontiguousarray(np.asarray(inputs["W"], dtype=np.float32))
    b = np.ascontiguousarray(np.asarray(inputs["bias"], dtype=np.float32))

    in_maps = make_in_maps(x, s, W, b)
    nc = _get_nc()
    res = run_bass_kernel_spmd(nc, in_maps, list(range(NCORES))).results
    # unpack (P, [c][j][CH]) per core -> (B, N)
    parts = []
    for c in range(NCORES):
        o = res[c]["outT"].astype(np.float32).reshape(P, NCH, KT, CH)
        parts.append(o.transpose(1, 3, 2, 0).reshape(BC, N))
    out = np.ascontiguousarray(DT * np.concatenate(parts, axis=0))
    return (out, out)


# revision 15
# speedup vs baseline: 1.0431x; 1.0168x over previous
"""CTRNN cell (6 Euler unfolds) on 8 Trainium2 NeuronCores.

Math (per unfold, 6x):
    f     = tanh([x, s] @ W + b)
    s_new = s + 0.1 * (-s + f)  = 0.9*s + 0.1*f

Strategy (v4 — chunk-pipelined bf16, TT-subtract critical path):
  - Data-parallel over batch: B=8192 -> 1024 rows/core, no cross-core
    communication. Host does the cheap numpy transposes/packing.
  - Everything transposed on-chip (feature dim on SBUF partitions, batch
    on the free dim); W slices are directly the stationary lhsT.
  - pre = x @ W_top computed once; per-unfold matmuls in *delta* form:
    one persistent PSUM accumulator per (m-tile, chunk) holds
    pre + s_k @ W_bot across all unfolds via psum += (f_k - s_k) @ (0.1*W_bot).
    7-logical-matmul FLOP floor, PSUM never restarts.
  - bf16 everywhere except PSUM and the f32 output: x, W, s, f, tmp.
    PSUM accumulation stays fp32. Errors are deterministic (fixed seed):
    ~2.6e-2 absmax vs the 6.3e-2 gate.
  - The state is plain s (bf16, unscaled) so the critical-path op
    tmp = f - s is a dual-pumped all-bf16 DVE tensor_tensor (334 ns per
    512-chunk; scalar_tensor_tensor does NOT dual-pump, measured).  The
    m3 subtract (the last-consumed k-tile, ~3us of slack) runs on GpSimd
    (~1.4us there; stt is ISA-illegal on Pool so the s += 0.1*tmp updates
    all stay on DVE). Per chunk-slot: PE 3.63us (pacer), ACT 2.5us,
    DVE ~3.4us, GpSimd ~1.4us.
  - Init is ONE 8-k-tile round per chunk alternating x/s k-tiles, so it
    consumes input pieces in exactly DMA-arrival order: x on the sync
    HWDGE ring, W_top/W_bot interleaved on the scalar ring, s on SWDGE,
    each ring issuing pieces in need order. 0.1*W_bot for the delta
    rounds is derived on-chip (ACT copy, scale=0.1) per landed piece.
  - Chunk pipelining: each 1024-batch round is split into two 512-batch
    chunk-rounds (c0/c1); tanh/subtract for chunk c0 run while the PE
    does chunk c1, so the tensor engine never waits at unfold bounds.
  - The final unfold is uniform with the others: the updated bf16 state
    itself is the output, DMA'd out per (m, chunk) piece as soon as its
    stt lands (1 MB total); the host just transposes/upcasts.
  - A minimal junk-matmul + junk-tanh warm-up starts the HAM un-throttle
    clock and pre-loads the ACT tanh table during the DMA lead-in.
"""

import numpy as np

UNFOLDS = 6
DT = 0.1
B, D, N = 8192, 512, 512
NCORES = 8
BC = B // NCORES          # batch rows per core
CH = 512                  # chunk: matmul moving free dim (1 PSUM bank)
NCH = BC // CH            # 2
P = 128
KT = D // P               # k-tiles (4) for each of W_top / W_bot
MT = N // P               # m-tiles of the output dim (4)

_compiled_nc = None


def _build_nc():
    import concourse.bass as bass  # noqa: F401
    import concourse.bacc as bacc
    import concourse.tile as tile
    from concourse import mybir

    f32 = mybir.dt.float32
    bf16 = mybir.dt.bfloat16
    MULT = mybir.AluOpType.mult
    ADD = mybir.AluOpType.add
    SUB = mybir.AluOpType.subtract
    TANH = mybir.ActivationFunctionType.Tanh

    nc = bacc.Bacc("TRN2", target_bir_lowering=False, debug=False)

    xB = nc.dram_tensor("xB", [P, KT * BC], bf16, kind="ExternalInput").ap()
    sB = nc.dram_tensor("sB", [P, KT * BC], bf16, kind="ExternalInput").ap()
    wB = nc.dram_tensor("wB", [P, 2 * KT * N], bf16, kind="ExternalInput").ap()
    bias = nc.dram_tensor("bias", [N], f32, kind="ExternalInput").ap()
    outT = nc.dram_tensor("outT", [P, KT * BC], bf16, kind="ExternalOutput").ap()

    with tile.TileContext(nc) as tc:
        with (
            tc.tile_pool(name="weights", bufs=1) as wpool,
            tc.tile_pool(name="data", bufs=1) as data,
            tc.tile_pool(name="tmp", bufs=2) as tmpp,
            tc.tile_pool(name="fpool", bufs=2) as fpool,
            tc.tile_pool(name="opool", bufs=1) as opool,
            tc.tile_pool(name="psum", bufs=1, space="PSUM") as psump,
        ):
            # ---- warm-up: start HAM clock + load the ACT tanh table ------
            junk = wpool.tile([P, 2 * P], bf16, tag="junk", name="junk")
            nc.vector.memset(junk[:], 0)
            junk2 = wpool.tile([P, 2 * P], bf16, tag="junk2", name="junk2")

            # ---- input DMAs: need-ordered pieces on 3 rings --------------
            x_sb = data.tile([P, KT * BC], bf16, tag="x", name="x_sb")
            v_sb = data.tile([P, KT * BC], bf16, tag="v", name="v_sb")
            w_sb = wpool.tile([P, 2 * KT * N], bf16, tag="w", name="w_sb")
            wb01_sb = wpool.tile([P, KT * N], bf16, tag="wb01", name="wb01_sb")
            bias_sb = wpool.tile([P, MT], f32, tag="bias", name="bias_sb")

            def xsl(j, c):
                # c-outer packing: chunk c occupies one contiguous half
                return slice(c * KT * CH + j * CH, c * KT * CH + (j + 1) * CH)

            HC = KT * CH  # elements per chunk-half (2048)
            # coarse 512 KB pieces; rings drain round-robin at packet
            # granularity, so the per-ring FIFO orders below give a global
            # arrival order matching init consumption:
            #   [x_c0, W_A, s_c0], [x_c1, W_B, bias], [s_c1]
            nc.sync.dma_start(x_sb[:, 0:HC], xB[:, 0:HC])
            nc.sync.dma_start(x_sb[:, HC:2 * HC], xB[:, HC:2 * HC])
            nc.sync.dma_start(v_sb[:, HC:2 * HC], sB[:, HC:2 * HC])
            nc.scalar.dma_start(w_sb[:, 0:HC], wB[:, 0:HC])
            nc.scalar.dma_start(w_sb[:, HC:2 * HC], wB[:, HC:2 * HC])
            nc.gpsimd.dma_start(v_sb[:, 0:HC], sB[:, 0:HC])
            nc.gpsimd.dma_start(bias_sb[:], bias.rearrange("(m p) -> p m", p=P))
            # junk tanh pre-loads the ACT tanh table (~2.7us) during the W
            # transfer; emitted AFTER the W dma_starts so it does not delay
            # their dispatch on the scalar sequencer.
            nc.scalar.activation(junk2[:], junk[:], TANH)
            # 0.1*W_bot for the delta rounds (ACT copy, scale=0.1)
            for j in range(KT):
                nc.scalar.mul(wb01_sb[:, j * N:(j + 1) * N],
                              w_sb[:, j * BC + CH: (j + 1) * BC], DT)

            # ---- persistent PSUM accumulators ----------------------------
            ps = [psump.tile([P, BC], f32, tag=f"ps{m}", name=f"ps{m}")
                  for m in range(MT)]

            # junk matmuls keep the PE busy (~3.4us continuous) so HAM
            # un-throttles to 2.4 GHz before the real matmuls stream.
            # 16 x 256-wide to 8 distinct half-banks: no WAW chains, so
            # they issue back-to-back; overwritten by the first start=True
            # real matmul per half-bank.
            for r in range(26):
                m, c = r % MT, (r // MT) % NCH
                nc.tensor.matmul(
                    ps[m][:, c * CH: c * CH + 2 * P],
                    lhsT=junk[:, 0:P], rhs=junk[:],
                    start=True, stop=True, skip_group_check=True,
                )

            # init: psum = x @ W_top + s0 @ W_bot, one 8-k-tile round per
            # chunk, alternating x/s k-tiles (DMA-arrival order)
            for c in range(NCH):
                INIT_ORDER = [("x", 0), ("x", 1), ("s", 0), ("s", 1),
                              ("x", 2), ("x", 3), ("s", 2), ("s", 3)]
                for t, (kind, j) in enumerate(INIT_ORDER):
                    off = j * BC + (0 if kind == "x" else CH)
                    rhs_t = x_sb if kind == "x" else v_sb
                    for m in range(MT):
                        nc.tensor.matmul(
                            ps[m][:, c * CH:(c + 1) * CH],
                            lhsT=w_sb[:, off + m * P: off + (m + 1) * P],
                            rhs=rhs_t[:, xsl(j, c)],
                            start=(t == 0),
                            stop=False,
                            skip_group_check=True,
                        )

            # ---- unfolds (chunk-pipelined) -------------------------------
            # Per chunk-round:
            #   f = tanh(psum + bias)            (ACT, 512 wide, bf16 out)
            #   tmp = f - s                      (TT sub: DVE m0/m1, GpSimd m2/m3)
            #   psum += tmp @ (0.1*W_bot)        (16 matmuls; skipped on k=5)
            #   s += 0.1*tmp                     (stt, DVE)
            # The last unfold is uniform: the updated bf16 state IS the
            # output, DMA'd per (m, chunk) piece (host just unpacks).
            out_eng = [nc.sync, nc.scalar]
            for k in range(UNFOLDS - 1):
                last = False
                tmp_t = [tmpp.tile([P, BC], bf16, tag=f"tmp{j}",
                                   name=f"tmp{k}_{j}")
                         for j in range(MT)]
                t01_t = [tmpp.tile([P, BC], bf16, tag=f"t01_{j}",
                                   name=f"t01_{k}_{j}")
                         for j in range(2)]
                f_t = [fpool.tile([P, BC], bf16, tag=f"f{m}", name=f"f{k}_{m}")
                       for m in range(MT)]
                for c in range(NCH):
                    cs = slice(c * CH, (c + 1) * CH)
                    for m in range(MT):
                        nc.scalar.activation(
                            f_t[m][:, cs], ps[m][:, cs], TANH,
                            bias=bias_sb[:, m:m + 1], scale=1.0,
                        )
                    for m in range(MT):
                        nc.vector.tensor_tensor(
                            tmp_t[m][:, cs], f_t[m][:, cs],
                            v_sb[:, xsl(m, c)], SUB,
                        )
                    # m-outer: ps[m] only written after tanh(m) freed it,
                    # and m0 closes early for the next slot's tanh chain
                    for m in range(MT):
                        for j in range(KT):
                            nc.tensor.matmul(
                                ps[m][:, cs],
                                lhsT=wb01_sb[:, j * N + m * P:
                                             j * N + (m + 1) * P],
                                rhs=tmp_t[j][:, cs],
                                start=False,
                                stop=(k == UNFOLDS - 2 and j == KT - 1),
                                skip_group_check=True,
                            )
                    # state update (off the critical path): ACT scales
                    # 0.1*tmp for m0/m1 (slack capacity there), making the
                    # add dual-pumped; m2/m3 use the 1x stt on DVE.
                    ncop = 1
                    for m in range(ncop):
                        nc.scalar.mul(t01_t[m][:, cs], tmp_t[m][:, cs], DT)
                        nc.vector.tensor_tensor(
                            v_sb[:, xsl(m, c)], v_sb[:, xsl(m, c)],
                            t01_t[m][:, cs], ADD,
                        )
                    for m in range(ncop, MT):
                        nc.vector.scalar_tensor_tensor(
                            v_sb[:, xsl(m, c)], tmp_t[m][:, cs], DT,
                            v_sb[:, xsl(m, c)], op0=MULT, op1=ADD,
                        )

            # final unfold: out10 = 9*s + f = 10*s_6 (bf16, in place over
            # the state), one strided DMA per chunk; host scales by 0.1.
            f_t = [fpool.tile([P, BC], bf16, tag=f"f{m}", name=f"f5_{m}")
                   for m in range(MT)]
            for c in range(NCH):
                cs = slice(c * CH, (c + 1) * CH)
                for m in range(MT):
                    nc.scalar.activation(
                        f_t[m][:, cs], ps[m][:, cs], TANH,
                        bias=bias_sb[:, m:m + 1], scale=1.0,
                    )
                for m in range(MT):
                    nc.vector.scalar_tensor_tensor(
                        v_sb[:, xsl(m, c)], v_sb[:, xsl(m, c)], 9.0,
                        f_t[m][:, cs], op0=MULT, op1=ADD,
                    )
                out_eng[c].dma_start(outT[:, c * HC:(c + 1) * HC],
                                     v_sb[:, c * HC:(c + 1) * HC])

    nc.compile()
    return nc


def _get_nc():
    global _compiled_nc
    if _compiled_nc is None:
        _compiled_nc = _build_nc()
    return _compiled_nc


def _couter_pack(a):
    """(KT*P, BC) -> (P, [c][j][CH]): chunk-outer, k-tiles inside."""
    return np.ascontiguousarray(
        a.reshape(KT, P, NCH, CH).transpose(1, 2, 0, 3).reshape(P, -1))


def make_in_maps(x, s, W, b):
    """Shard + pack host-side. x/W/s in bf16; all packed as (128, KT*free)
    k-tile layouts so every DMA piece has contiguous per-partition runs."""
    import ml_dtypes
    bf16 = ml_dtypes.bfloat16

    xT = np.ascontiguousarray(x.T)            # (D, B)
    sT = np.ascontiguousarray(s.T)            # (N, B)
    # W interleaved: (P, [j][Wt_j(512) | Wb_j(512)])
    Wt = W[:D].astype(bf16).reshape(KT, P, 1, N)
    Wb = W[D:].astype(bf16).reshape(KT, P, 1, N)
    Wi = np.ascontiguousarray(
        np.concatenate([Wt, Wb], axis=2).transpose(1, 0, 2, 3).reshape(P, -1))
    in_maps = []
    for c in range(NCORES):
        sl = slice(c * BC, (c + 1) * BC)
        in_maps.append({
            "xB": _couter_pack(xT[:, sl].astype(bf16)),
            "sB": _couter_pack(sT[:, sl].astype(bf16)),
            "wB": Wi,
            "bias": b,
        })
    return in_maps


def kernel(**inputs):
    from concourse.bass_utils import run_bass_kernel_spmd

    x = np.asarray(inputs["inputs"], dtype=np.float32)
    s = np.asarray(inputs["state"], dtype=np.float32)
    W = np.ascontiguousarray(np.asarray(inputs["W"], dtype=np.float32))
    b = np.ascontiguousarray(np.asarray(inputs["bias"], dtype=np.float32))

    in_maps = make_in_maps(x, s, W, b)
    nc = _get_nc()
    res = run_bass_kernel_spmd(nc, in_maps, list(range(NCORES))).results
    # unpack (P, [c][j][CH]) per core -> (B, N)
    parts = []
    for c in range(NCORES):
        o = res[c]["outT"].astype(np.float32).reshape(P, NCH, KT, CH)
        parts.append(o.transpose(1, 3, 2, 0).reshape(BC, N))
    out = np.ascontiguousarray(DT * np.concatenate(parts, axis=0))
    return (out, out)
